# revision 5
# baseline (speedup 1.0000x reference)
"""Multi-head causal attention (B=4, T=2048, C=1024, H=16) on 8 NeuronCores.

Sharding: data-parallel over batch (4 batches x 2 cores) with an even/odd
query interleave inside each batch so causal work per core is uniform and one
SPMD program serves all cores. The host hands each core its batch with tokens
permuted to [own-parity tokens | other-parity tokens], so the core's 1024
queries are the first half of the token axis and parity differences live only
in input data (token order + causal masks). Each core:
  - computes K^T and V for all 2048 permuted tokens and Q^T for its 1024
    queries with fp32r matmuls,
  - runs causal attention for all 16 heads in transposed [keys, queries]
    score layout; the softmax denominator comes from augmenting V with an
    all-ones column block so the AV matmul emits it pre-broadcast on
    partitions 64..127,
  - projects with Wp, adds the bias, and writes a compact [1024, 1024] output
    the host scatters back to out[b, parity::2, :].
"""
import sys

sys.path.insert(0, "/opt/trn_rl_repo")

from contextlib import ExitStack

import numpy as np

import concourse.bass as bass
import concourse.mybir as mybir
from concourse import tile
from concourse.bass_utils import run_bass_kernel_spmd

F32 = mybir.dt.float32
F32R = mybir.dt.float32r
AF = mybir.ActivationFunctionType

B, T, C, H, D = 4, 2048, 1024, 16, 64
TQ = T // 2          # queries per core
P = 128
CCH = C // P         # 8 contraction chunks
NKB = T // P         # 16 key blocks (permuted order)
NJ = TQ // 256       # 4 query chunks of 256
QH = 4               # heads per quarter-phase
NQP = H // QH        # 4 quarter-phases
NEG = -1e9

LAST_RESULT = None

_waitsplit_counter = [0]


def _legalize_waits(nc):
    """walrus in this container accepts at most 1 sync wait per instruction
    (2 for EventSemaphore). Hoist extras onto same-engine NoOps placed just
    before the instruction; the in-order sequencer keeps semantics."""
    n_fixed = 0
    for fn in nc.m.functions:
        for blk in fn.blocks:
            insts = blk.instructions
            i = 0
            while i < len(insts):
                inst = insts[i]
                si = inst.sync_info
                cap = 2 if isinstance(inst, mybir.InstEventSemaphore) else 1
                if si is not None and len(si.on_wait) > cap:
                    extra = list(si.on_wait[:-cap])
                    si.on_wait = list(si.on_wait[-cap:])
                    for w in extra:
                        _waitsplit_counter[0] += 1
                        nop = mybir.InstNoOp(
                            name=f"I-waitsplit-{_waitsplit_counter[0]}",
                            engine=inst.engine,
                            ins=[],
                            outs=[],
                            sync_info=mybir.SyncInfo(on_wait=[w], on_update=[]),
                        )
                        insts.insert(i, nop)
                        i += 1
                    n_fixed += 1
                i += 1
    return n_fixed


def _group_blocks(G):
    """Key blocks of accumulation group G: a same-parity pair and the
    matching other-parity pair (other-parity tokens start at block 8)."""
    return [2 * G, 2 * G + 1, 8 + 2 * G, 9 + 2 * G]


def _build():
    nc = bass.Bass(trn_type="TRN2", target_bir_lowering=False)
    xT = nc.dram_tensor("xT", (C, T), F32, kind="ExternalInput")
    wq = nc.dram_tensor("wq", (C, C), F32, kind="ExternalInput")
    wk = nc.dram_tensor("wk", (C, C), F32, kind="ExternalInput")
    wv = nc.dram_tensor("wv", (C, C), F32, kind="ExternalInput")
    wp = nc.dram_tensor("wp", (C, C), F32, kind="ExternalInput")
    bpb = nc.dram_tensor("bpb", (P, C), F32, kind="ExternalInput")
    maskd = nc.dram_tensor("mask", (P, 4 * 256), F32, kind="ExternalInput")
    out = nc.dram_tensor("out", (TQ, C), F32, kind="ExternalOutput")

    with tile.TileContext(nc) as tc, ExitStack() as ctx:
        res = ctx.enter_context(tc.tile_pool(name="res", bufs=1))
        dram = ctx.enter_context(tc.tile_pool(name="dram", bufs=1, space="DRAM"))

        # ---- resident loads: xT (rounded to f32r), mask, bias, ones ----
        xtr = []
        with tc.tile_pool(name="xload", bufs=2) as xload:
            for cc in range(CCH):
                st = xload.tile([P, T], F32, tag="xstage")
                nc.sync.dma_start(st[:], xT.ap()[cc * P:(cc + 1) * P, :])
                xr = res.tile([P, T], F32R, tag=f"xtr{cc}")
                nc.scalar.copy(xr[:], st[:])
                xtr.append(xr)
        mask_sb = res.tile([P, 4, 256], F32, tag="mask")
        nc.sync.dma_start(mask_sb[:], maskd.ap().rearrange("p (m a) -> p m a", m=4))
        bp_sb = res.tile([P, C], F32, tag="bpb")
        nc.sync.dma_start(bp_sb[:], bpb.ap())
        ones_sb = res.tile([P, 64], F32, tag="ones")
        nc.vector.memset(ones_sb[:], 1.0)

        yt_dram = dram.tile([C, TQ], F32)

        # ---- quarter-phases: 4 heads each ----
        for qq in range(NQP):
            with ExitStack() as qctx:
                qpool = qctx.enter_context(tc.tile_pool(name=f"q{qq}", bufs=1))
                wpool = qctx.enter_context(tc.tile_pool(name=f"w{qq}", bufs=2))
                wstage = qctx.enter_context(tc.tile_pool(name=f"ws{qq}", bufs=2))

                # weight slices for this quarter (columns 256*qq .. +256);
                # wk/wv/wq share the per-cc slots (sequential phases)
                def load_w(wt):
                    slices = []
                    for cc in range(CCH):
                        st = wstage.tile([P, 256], F32, tag="wstage")
                        nc.sync.dma_start(
                            st[:],
                            wt.ap()[cc * P:(cc + 1) * P,
                                    qq * 256:(qq + 1) * 256])
                        wr = wpool.tile([P, 256], F32R, tag=f"w{cc}")
                        nc.scalar.copy(wr[:], st[:])
                        slices.append(wr)
                    return slices

                ktq = [qpool.tile([P, T], F32R, tag=f"ktq{dc}",
                                  name=f"ktq{qq}_{dc}")
                       for dc in range(2)]
                qtq = [qpool.tile([P, TQ], F32R, tag=f"qtq{dc}",
                                  name=f"qtq{qq}_{dc}")
                       for dc in range(2)]
                vt = [qpool.tile([P, QH, P], F32R, tag=f"vt{tt}",
                                 name=f"vt{qq}_{tt}")
                      for tt in range(NKB)]

                with tc.tile_pool(name=f"pps{qq}", bufs=4, space="PSUM") as pps:
                    # K^T projection: [256 dcols, 2048 tok]
                    wkr = load_w(wk)
                    for dc in range(2):
                        for tt in range(4):
                            ps = pps.tile([P, 512], F32, tag="proj")
                            for cc in range(CCH):
                                nc.tensor.matmul(
                                    ps[:],
                                    wkr[cc][:, dc * P:(dc + 1) * P],
                                    xtr[cc][:, tt * 512:(tt + 1) * 512],
                                    start=(cc == 0), stop=(cc == CCH - 1))
                            nc.vector.tensor_copy(
                                ktq[dc][:, tt * 512:(tt + 1) * 512], ps[:])
                    # V projection: [2048 tok, 256 dv] -> vt tiles + ones
                    wvr = load_w(wv)
                    for tt in range(NKB):
                        ps = pps.tile([P, 256], F32, tag="vproj")
                        for cc in range(CCH):
                            nc.tensor.matmul(
                                ps[:],
                                xtr[cc][:, tt * P:(tt + 1) * P],
                                wvr[cc][:],
                                start=(cc == 0), stop=(cc == CCH - 1))
                        nc.vector.tensor_copy(
                            vt[tt][:, :, 0:64],
                            ps[:].rearrange("t (h d) -> t h d", h=QH))
                        nc.vector.tensor_copy(
                            vt[tt][:, :, 64:128],
                            ones_sb[:, None, :].to_broadcast((P, QH, 64)))
                    # Q^T projection: queries are the first TQ tokens
                    wqr = load_w(wq)
                    for dc in range(2):
                        for qt in range(2):
                            ps = pps.tile([P, 512], F32, tag="proj")
                            for cc in range(CCH):
                                nc.tensor.matmul(
                                    ps[:],
                                    wqr[cc][:, dc * P:(dc + 1) * P],
                                    xtr[cc][:, qt * 512:(qt + 1) * 512],
                                    start=(cc == 0), stop=(cc == CCH - 1))
                            nc.vector.tensor_copy(
                                qtq[dc][:, qt * 512:(qt + 1) * 512], ps[:])

                # attention for the 2 head-pairs of this quarter
                with tc.tile_pool(name=f"aps{qq}", bufs=2, space="PSUM") as sps_pool, \
                     tc.tile_pool(name=f"yps{qq}", bufs=4, space="PSUM") as yps_pool, \
                     tc.tile_pool(name=f"pt{qq}", bufs=3) as pt_pool, \
                     tc.tile_pool(name=f"nrm{qq}", bufs=2) as nrm_pool:
                    for hp in range(2):
                        for J in range(NJ):
                            y_ps = [yps_pool.tile([P, 256], F32, tag="yps",
                                                  name=f"yps{qq}_{hp}_{J}_{e}")
                                    for e in range(2)]
                            for G in range(J + 1):
                                blocks = _group_blocks(G)
                                pts = []
                                for e in range(2):
                                    s_ps = sps_pool.tile([P, 4, 256], F32,
                                                         tag="sps")
                                    for m, kb in enumerate(blocks):
                                        nc.tensor.matmul(
                                            s_ps[:, m, :],
                                            ktq[hp][64 * e:64 * e + 64,
                                                    kb * P:(kb + 1) * P],
                                            qtq[hp][64 * e:64 * e + 64,
                                                    J * 256:(J + 1) * 256],
                                            start=True, stop=True)
                                    if G == J:
                                        nc.vector.tensor_add(
                                            s_ps[:], s_ps[:], mask_sb[:])
                                    pt = pt_pool.tile([P, 4, 256], F32R,
                                                      tag="pt")
                                    nc.scalar.activation(
                                        pt[:], s_ps[:], AF.Exp, scale=0.125)
                                    pts.append(pt)
                                for e in range(2):
                                    for m, kb in enumerate(blocks):
                                        nc.tensor.matmul(
                                            y_ps[e][:],
                                            vt[kb][:, 2 * hp + e, :],
                                            pts[e][:, m, :],
                                            start=(G == 0 and m == 0),
                                            stop=(G == J and m == 3))
                            ystage = nrm_pool.tile([P, 256], F32, tag="ystage")
                            for e in range(2):
                                rec = nrm_pool.tile([P, 256], F32, tag="rec")
                                nc.vector.reciprocal(
                                    rec[0:64, :], y_ps[e][64:128, :])
                                nc.vector.tensor_mul(
                                    ystage[64 * e:64 * e + 64, :],
                                    y_ps[e][0:64, :], rec[0:64, :])
                            row = P * (2 * qq + hp)
                            nc.sync.dma_start(
                                yt_dram[row:row + P, J * 256:(J + 1) * 256],
                                ystage[:])

        # ---- output projection: out = y @ Wp + bp ----
        with ExitStack() as octx:
            opool = octx.enter_context(tc.tile_pool(name="oproj", bufs=1))
            ostg = octx.enter_context(tc.tile_pool(name="ostg", bufs=3))
            ops = octx.enter_context(
                tc.tile_pool(name="ops", bufs=4, space="PSUM"))
            ytr = []
            wpr = []
            for hp in range(8):
                st = ostg.tile([P, TQ], F32, tag="ytstage")
                nc.sync.dma_start(st[:], yt_dram[hp * P:(hp + 1) * P, :])
                yr = opool.tile([P, TQ], F32R, tag=f"ytr{hp}")
                nc.scalar.copy(yr[:], st[:])
                ytr.append(yr)
                st2 = ostg.tile([P, C], F32, tag="wpstage")
                nc.sync.dma_start(st2[:], wp.ap()[hp * P:(hp + 1) * P, :])
                wr = opool.tile([P, C], F32R, tag=f"wpr{hp}")
                nc.scalar.copy(wr[:], st2[:])
                wpr.append(wr)
            for qc in range(8):
                for oc in range(2):
                    ps = ops.tile([P, 512], F32, tag="ops")
                    for hp in range(8):
                        nc.tensor.matmul(
                            ps[:],
                            ytr[hp][:, qc * P:(qc + 1) * P],
                            wpr[hp][:, oc * 512:(oc + 1) * 512],
                            start=(hp == 0), stop=(hp == 7))
                    ostage = ostg.tile([P, 512], F32, tag="ostage")
                    nc.vector.tensor_add(
                        ostage[:], ps[:], bp_sb[:, oc * 512:(oc + 1) * 512])
                    nc.sync.dma_start(
                        out.ap()[qc * P:(qc + 1) * P,
                                 oc * 512:(oc + 1) * 512],
                        ostage[:])

    _legalize_waits(nc)
    return nc


_nc_cache = []


def _host_masks():
    """Additive causal masks [P, 4, 256] per parity, for the diagonal
    accumulation group (same-parity pair then other-parity pair)."""
    masks = {}
    cc = np.arange(P)[:, None]
    aa = np.arange(256)[None, :]
    for p in range(2):
        m = np.empty((P, 4, 256), np.float32)
        for mm in range(2):       # same-parity blocks
            m[:, mm, :] = np.where(128 * mm + cc <= aa, 0.0, NEG)
        for mm in range(2):       # other-parity blocks
            m[:, 2 + mm, :] = np.where(128 * mm + cc <= aa - 1 + p, 0.0, NEG)
        masks[p] = m.reshape(P, 4 * 256)
    return masks


def kernel(x, Wq, Wk, Wv, Wp, bp):
    global LAST_RESULT
    x = np.asarray(x, np.float32)
    Wq = np.asarray(Wq, np.float32)
    Wk = np.asarray(Wk, np.float32)
    Wv = np.asarray(Wv, np.float32)
    Wp = np.asarray(Wp, np.float32)
    bp = np.asarray(bp, np.float32)

    if not _nc_cache:
        _nc_cache.append(_build())
    nc = _nc_cache[0]

    bp_bc = np.ascontiguousarray(np.broadcast_to(bp[None, :], (P, C)))
    masks = _host_masks()

    in_maps = []
    for c in range(8):
        b, p = divmod(c, 2)
        xperm = np.concatenate([x[b, p::2, :], x[b, 1 - p::2, :]], axis=0)
        in_maps.append({
            "xT": np.ascontiguousarray(xperm.T),
            "wq": Wq, "wk": Wk, "wv": Wv, "wp": Wp,
            "bpb": bp_bc,
            "mask": masks[p],
        })

    LAST_RESULT = run_bass_kernel_spmd(nc, in_maps, core_ids=list(range(8)))
    out = np.empty((B, T, C), np.float32)
    for c in range(8):
        b, p = divmod(c, 2)
        out[b, p::2, :] = LAST_RESULT.results[c]["out"]
    return out


# revision 6
# speedup vs baseline: 10.4393x; 10.4393x over previous
"""Multi-head causal attention (B=4, T=2048, C=1024, H=16) on 8 NeuronCores.

Sharding: data-parallel over batch (4 batches x 2 cores) with an even/odd
query interleave inside each batch so causal work per core is uniform and one
SPMD program serves all cores. The host hands each core its batch with tokens
permuted to [own-parity tokens | other-parity tokens], so the core's 1024
queries are the first half of the token axis and parity differences live only
in input data (token order + causal masks). Each core:
  - computes K^T and V for all 2048 permuted tokens and Q^T for its 1024
    queries with fp32r matmuls,
  - runs causal attention for all 16 heads in transposed [keys, queries]
    score layout; the softmax denominator comes from augmenting V with an
    all-ones column block so the AV matmul emits it pre-broadcast on
    partitions 64..127,
  - projects with Wp, adds the bias, and writes a compact [1024, 1024] output
    the host scatters back to out[b, parity::2, :].
"""
import sys

sys.path.insert(0, "/opt/trn_rl_repo")

from contextlib import ExitStack

import numpy as np

import concourse.bass as bass
import concourse.mybir as mybir
from concourse import tile
from concourse.bass_utils import run_bass_kernel_spmd

F32 = mybir.dt.float32
F32R = mybir.dt.float32r
AF = mybir.ActivationFunctionType

B, T, C, H, D = 4, 2048, 1024, 16, 64
TQ = T // 2          # queries per core
P = 128
CCH = C // P         # 8 contraction chunks
NKB = T // P         # 16 key blocks (permuted order)
NJ = TQ // 256       # 4 query chunks of 256
QH = 4               # heads per quarter-phase
NQP = H // QH        # 4 quarter-phases
NEG = -1e9

LAST_RESULT = None

_waitsplit_counter = [0]


def _legalize_waits(nc):
    """walrus in this container accepts at most 1 sync wait per instruction
    (2 for EventSemaphore). Hoist extras onto same-engine NoOps placed just
    before the instruction; the in-order sequencer keeps semantics."""
    n_fixed = 0
    for fn in nc.m.functions:
        for blk in fn.blocks:
            insts = blk.instructions
            i = 0
            while i < len(insts):
                inst = insts[i]
                si = inst.sync_info
                cap = 2 if isinstance(inst, mybir.InstEventSemaphore) else 1
                if si is not None and len(si.on_wait) > cap:
                    extra = list(si.on_wait[:-cap])
                    si.on_wait = list(si.on_wait[-cap:])
                    for w in extra:
                        _waitsplit_counter[0] += 1
                        nop = mybir.InstNoOp(
                            name=f"I-waitsplit-{_waitsplit_counter[0]}",
                            engine=inst.engine,
                            ins=[],
                            outs=[],
                            sync_info=mybir.SyncInfo(on_wait=[w], on_update=[]),
                        )
                        insts.insert(i, nop)
                        i += 1
                    n_fixed += 1
                i += 1
    return n_fixed


def _group_blocks(G):
    """Key blocks of accumulation group G: a same-parity pair and the
    matching other-parity pair (other-parity tokens start at block 8)."""
    return [2 * G, 2 * G + 1, 8 + 2 * G, 9 + 2 * G]


def _build(repeat=1):
    nc = bass.Bass(trn_type="TRN2", target_bir_lowering=False)
    xT = nc.dram_tensor("xT", (C, T), F32, kind="ExternalInput")
    wq = nc.dram_tensor("wq", (C, C), F32, kind="ExternalInput")
    wk = nc.dram_tensor("wk", (C, C), F32, kind="ExternalInput")
    wv = nc.dram_tensor("wv", (C, C), F32, kind="ExternalInput")
    wp = nc.dram_tensor("wp", (C, C), F32, kind="ExternalInput")
    bpb = nc.dram_tensor("bpb", (P, C), F32, kind="ExternalInput")
    maskd = nc.dram_tensor("mask", (P, 4 * 256), F32, kind="ExternalInput")
    out = nc.dram_tensor("out", (TQ, C), F32, kind="ExternalOutput")

    with tile.TileContext(nc) as tc, ExitStack() as ctx:
        res = ctx.enter_context(tc.tile_pool(name="res", bufs=1))
        dram = ctx.enter_context(tc.tile_pool(name="dram", bufs=1, space="DRAM"))

        # ---- resident loads: xT (rounded to f32r), mask, bias, ones ----
        xtr = []
        with tc.tile_pool(name="xload", bufs=2) as xload:
            for cc in range(CCH):
                st = xload.tile([P, T], F32, tag="xstage")
                nc.sync.dma_start(st[:], xT.ap()[cc * P:(cc + 1) * P, :])
                xr = res.tile([P, T], F32R, tag=f"xtr{cc}")
                nc.scalar.copy(xr[:], st[:])
                xtr.append(xr)
        mask_sb = res.tile([P, 4, 256], F32, tag="mask")
        nc.sync.dma_start(mask_sb[:], maskd.ap().rearrange("p (m a) -> p m a", m=4))
        bp_sb = res.tile([P, C], F32, tag="bpb")
        nc.sync.dma_start(bp_sb[:], bpb.ap())
        ones_sb = res.tile([P, 64], F32, tag="ones")
        nc.vector.memset(ones_sb[:], 1.0)

        yt_dram = dram.tile([C, TQ], F32)

        rep_ctx = ExitStack()
        if repeat > 1:
            rep_ctx.enter_context(tc.For_i(0, repeat, 1))

        # ---- quarter-phases: 4 heads each ----
        for qq in range(NQP):
            with ExitStack() as qctx:
                qpool = qctx.enter_context(tc.tile_pool(name=f"q{qq}", bufs=1))
                wpool = qctx.enter_context(tc.tile_pool(name=f"w{qq}", bufs=2))
                wstage = qctx.enter_context(tc.tile_pool(name=f"ws{qq}", bufs=2))

                # weight slices for this quarter (columns 256*qq .. +256);
                # wk/wv/wq share the per-cc slots (sequential phases)
                def load_w(wt):
                    slices = []
                    for cc in range(CCH):
                        st = wstage.tile([P, 256], F32, tag="wstage")
                        nc.sync.dma_start(
                            st[:],
                            wt.ap()[cc * P:(cc + 1) * P,
                                    qq * 256:(qq + 1) * 256])
                        wr = wpool.tile([P, 256], F32R, tag=f"w{cc}")
                        nc.scalar.copy(wr[:], st[:])
                        slices.append(wr)
                    return slices

                ktq = [qpool.tile([P, T], F32R, tag=f"ktq{dc}",
                                  name=f"ktq{qq}_{dc}")
                       for dc in range(2)]
                qtq = [qpool.tile([P, TQ], F32R, tag=f"qtq{dc}",
                                  name=f"qtq{qq}_{dc}")
                       for dc in range(2)]
                vt = [qpool.tile([P, QH, P], F32R, tag=f"vt{tt}",
                                 name=f"vt{qq}_{tt}")
                      for tt in range(NKB)]

                with tc.tile_pool(name=f"pps{qq}", bufs=4, space="PSUM") as pps:
                    # K^T projection: [256 dcols, 2048 tok]
                    wkr = load_w(wk)
                    for dc in range(2):
                        for tt in range(4):
                            ps = pps.tile([P, 512], F32, tag="proj")
                            for cc in range(CCH):
                                nc.tensor.matmul(
                                    ps[:],
                                    wkr[cc][:, dc * P:(dc + 1) * P],
                                    xtr[cc][:, tt * 512:(tt + 1) * 512],
                                    start=(cc == 0), stop=(cc == CCH - 1))
                            nc.vector.tensor_copy(
                                ktq[dc][:, tt * 512:(tt + 1) * 512], ps[:])
                    # V projection: [2048 tok, 256 dv] -> vt tiles + ones
                    wvr = load_w(wv)
                    for tt in range(NKB):
                        ps = pps.tile([P, 256], F32, tag="vproj")
                        for cc in range(CCH):
                            nc.tensor.matmul(
                                ps[:],
                                xtr[cc][:, tt * P:(tt + 1) * P],
                                wvr[cc][:],
                                start=(cc == 0), stop=(cc == CCH - 1))
                        nc.vector.tensor_copy(
                            vt[tt][:, :, 0:64],
                            ps[:].rearrange("t (h d) -> t h d", h=QH))
                        nc.vector.tensor_copy(
                            vt[tt][:, :, 64:128],
                            ones_sb[:, None, :].to_broadcast((P, QH, 64)))
                    # Q^T projection: queries are the first TQ tokens
                    wqr = load_w(wq)
                    for dc in range(2):
                        for qt in range(2):
                            ps = pps.tile([P, 512], F32, tag="proj")
                            for cc in range(CCH):
                                nc.tensor.matmul(
                                    ps[:],
                                    wqr[cc][:, dc * P:(dc + 1) * P],
                                    xtr[cc][:, qt * 512:(qt + 1) * 512],
                                    start=(cc == 0), stop=(cc == CCH - 1))
                            nc.vector.tensor_copy(
                                qtq[dc][:, qt * 512:(qt + 1) * 512], ps[:])

                # attention for the 2 head-pairs of this quarter
                with tc.tile_pool(name=f"aps{qq}", bufs=2, space="PSUM") as sps_pool, \
                     tc.tile_pool(name=f"yps{qq}", bufs=4, space="PSUM") as yps_pool, \
                     tc.tile_pool(name=f"pt{qq}", bufs=3) as pt_pool, \
                     tc.tile_pool(name=f"nrm{qq}", bufs=2) as nrm_pool:
                    for hp in range(2):
                        for J in range(NJ):
                            y_ps = [yps_pool.tile([P, 256], F32, tag="yps",
                                                  name=f"yps{qq}_{hp}_{J}_{e}")
                                    for e in range(2)]
                            for G in range(J + 1):
                                blocks = _group_blocks(G)
                                pts = []
                                for e in range(2):
                                    s_ps = sps_pool.tile([P, 4, 256], F32,
                                                         tag="sps")
                                    for m, kb in enumerate(blocks):
                                        nc.tensor.matmul(
                                            s_ps[:, m, :],
                                            ktq[hp][64 * e:64 * e + 64,
                                                    kb * P:(kb + 1) * P],
                                            qtq[hp][64 * e:64 * e + 64,
                                                    J * 256:(J + 1) * 256],
                                            start=True, stop=True)
                                    if G == J:
                                        nc.vector.tensor_add(
                                            s_ps[:], s_ps[:], mask_sb[:])
                                    pt = pt_pool.tile([P, 4, 256], F32R,
                                                      tag="pt")
                                    nc.scalar.activation(
                                        pt[:], s_ps[:], AF.Exp, scale=0.125)
                                    pts.append(pt)
                                for e in range(2):
                                    for m, kb in enumerate(blocks):
                                        nc.tensor.matmul(
                                            y_ps[e][:],
                                            vt[kb][:, 2 * hp + e, :],
                                            pts[e][:, m, :],
                                            start=(G == 0 and m == 0),
                                            stop=(G == J and m == 3))
                            ystage = nrm_pool.tile([P, 256], F32, tag="ystage")
                            for e in range(2):
                                rec = nrm_pool.tile([P, 256], F32, tag="rec")
                                nc.vector.reciprocal(
                                    rec[0:64, :], y_ps[e][64:128, :])
                                nc.vector.tensor_mul(
                                    ystage[64 * e:64 * e + 64, :],
                                    y_ps[e][0:64, :], rec[0:64, :])
                            row = P * (2 * qq + hp)
                            nc.sync.dma_start(
                                yt_dram[row:row + P, J * 256:(J + 1) * 256],
                                ystage[:])

        # ---- output projection: out = y @ Wp + bp ----
        with ExitStack() as octx:
            opool = octx.enter_context(tc.tile_pool(name="oproj", bufs=1))
            ostg = octx.enter_context(tc.tile_pool(name="ostg", bufs=3))
            ops = octx.enter_context(
                tc.tile_pool(name="ops", bufs=4, space="PSUM"))
            ytr = []
            wpr = []
            for hp in range(8):
                st = ostg.tile([P, TQ], F32, tag="ytstage")
                nc.sync.dma_start(st[:], yt_dram[hp * P:(hp + 1) * P, :])
                yr = opool.tile([P, TQ], F32R, tag=f"ytr{hp}")
                nc.scalar.copy(yr[:], st[:])
                ytr.append(yr)
                st2 = ostg.tile([P, C], F32, tag="wpstage")
                nc.sync.dma_start(st2[:], wp.ap()[hp * P:(hp + 1) * P, :])
                wr = opool.tile([P, C], F32R, tag=f"wpr{hp}")
                nc.scalar.copy(wr[:], st2[:])
                wpr.append(wr)
            for qc in range(8):
                for oc in range(2):
                    ps = ops.tile([P, 512], F32, tag="ops")
                    for hp in range(8):
                        nc.tensor.matmul(
                            ps[:],
                            ytr[hp][:, qc * P:(qc + 1) * P],
                            wpr[hp][:, oc * 512:(oc + 1) * 512],
                            start=(hp == 0), stop=(hp == 7))
                    ostage = ostg.tile([P, 512], F32, tag="ostage")
                    nc.vector.tensor_add(
                        ostage[:], ps[:], bp_sb[:, oc * 512:(oc + 1) * 512])
                    nc.sync.dma_start(
                        out.ap()[qc * P:(qc + 1) * P,
                                 oc * 512:(oc + 1) * 512],
                        ostage[:])
        rep_ctx.close()

    _legalize_waits(nc)
    return nc


_nc_cache = []


def _host_masks():
    """Additive causal masks [P, 4, 256] per parity, for the diagonal
    accumulation group (same-parity pair then other-parity pair)."""
    masks = {}
    cc = np.arange(P)[:, None]
    aa = np.arange(256)[None, :]
    for p in range(2):
        m = np.empty((P, 4, 256), np.float32)
        for mm in range(2):       # same-parity blocks
            m[:, mm, :] = np.where(128 * mm + cc <= aa, 0.0, NEG)
        for mm in range(2):       # other-parity blocks
            m[:, 2 + mm, :] = np.where(128 * mm + cc <= aa - 1 + p, 0.0, NEG)
        masks[p] = m.reshape(P, 4 * 256)
    return masks


def kernel(x, Wq, Wk, Wv, Wp, bp):
    global LAST_RESULT
    x = np.asarray(x, np.float32)
    Wq = np.asarray(Wq, np.float32)
    Wk = np.asarray(Wk, np.float32)
    Wv = np.asarray(Wv, np.float32)
    Wp = np.asarray(Wp, np.float32)
    bp = np.asarray(bp, np.float32)

    if not _nc_cache:
        _nc_cache.append(_build())
    nc = _nc_cache[0]

    bp_bc = np.ascontiguousarray(np.broadcast_to(bp[None, :], (P, C)))
    masks = _host_masks()

    in_maps = []
    for c in range(8):
        b, p = divmod(c, 2)
        xperm = np.concatenate([x[b, p::2, :], x[b, 1 - p::2, :]], axis=0)
        in_maps.append({
            "xT": np.ascontiguousarray(xperm.T),
            "wq": Wq, "wk": Wk, "wv": Wv, "wp": Wp,
            "bpb": bp_bc,
            "mask": masks[p],
        })

    LAST_RESULT = run_bass_kernel_spmd(nc, in_maps, core_ids=list(range(8)))
    out = np.empty((B, T, C), np.float32)
    for c in range(8):
        b, p = divmod(c, 2)
        out[b, p::2, :] = LAST_RESULT.results[c]["out"]
    return out
